# revision 6
# baseline (speedup 1.0000x reference)
"""CP/PARAFAC bilinear regression kernel for Trainium2 (8 NeuronCores).

Computes y[n] = beta_0 + sum_{i,j} x[n,i,j] * w[i,j],  w = gamma^T @ alpha.

Sharding: data-parallel over the batch axis — each of the 8 cores gets
131072/8 = 16384 rows of x; the tiny gamma/alpha/beta_0 params are
replicated. Per core:
  - w is built on-device with one PE matmul (K padded to 128), bounced
    through a DRAM scratch and broadcast-DMA'd to all 128 partitions.
  - x rows are mapped rows p*128+t -> partition p, column t, so the row
    dot-products land in an SBUF tile y_sb[128,128] whose final DMA to
    DRAM is fully contiguous.
  - Each [128, 448] row-block is processed by ONE fused DVE op
    (scalar_tensor_tensor: out = (x*1)*w_rep, accum_out = row sums),
    i.e. a single pass over the data on the vector engine.
"""

import numpy as np

N_TOTAL = 131072
N_CORES = 8
N_PER_CORE = N_TOTAL // N_CORES  # 16384
NG = 7
NA = 64
RANK = 64
D = NG * NA  # 448
P = 128
N_TILES = N_PER_CORE // P  # 128 row-blocks -> columns of y_sb
T = 8  # row-blocks per DMA chunk (128*8*448*4B = 1.79 MiB per DMA)
N_CHUNKS = N_TILES // T  # 16

_CACHE = {}


def _build():
    from concourse import bacc, mybir, tile

    f32 = mybir.dt.float32

    nc = bacc.Bacc("TRN2", target_bir_lowering=False, debug=False)

    x_d = nc.dram_tensor("x", [N_PER_CORE, D], f32, kind="ExternalInput").ap()
    gamma_d = nc.dram_tensor("gamma", [RANK, NG], f32, kind="ExternalInput").ap()
    alpha_d = nc.dram_tensor("alpha", [RANK, NA], f32, kind="ExternalInput").ap()
    beta_d = nc.dram_tensor("beta", [1], f32, kind="ExternalInput").ap()
    y_d = nc.dram_tensor("y", [N_PER_CORE], f32, kind="ExternalOutput").ap()
    w_d = nc.dram_tensor("w_scratch", [D], f32).ap()

    mult = mybir.AluOpType.mult

    with tile.TileContext(nc) as tc:
        with (
            tc.tile_pool(name="const", bufs=1) as cpool,
            tc.tile_pool(name="xp", bufs=4) as xpool,
            tc.tile_pool(name="sc", bufs=2) as scpool,
            tc.tile_pool(name="ps", bufs=1, space="PSUM") as pspool,
        ):
            # ---- build w = gamma^T @ alpha on the PE (K = RANK = 64)
            g_sb = cpool.tile([RANK, NG], f32)
            a_sb = cpool.tile([RANK, NA], f32)
            nc.sync.dma_start(out=g_sb[:], in_=gamma_d[:, :])
            nc.sync.dma_start(out=a_sb[:], in_=alpha_d[:, :])
            # stage through DVE so the PE matmul needs only ONE sem wait
            # (PE Matmult has a single sync-wait slot in codegen)
            ga_sb = cpool.tile([RANK, NG + NA], f32)
            nc.vector.tensor_copy(out=ga_sb[:, :NG], in_=g_sb[:])
            nc.vector.tensor_copy(out=ga_sb[:, NG:], in_=a_sb[:])
            w_ps = pspool.tile([NG, NA], f32)
            nc.tensor.matmul(
                w_ps[:], ga_sb[:, :NG], ga_sb[:, NG:], start=True, stop=True
            )
            w_sb = cpool.tile([NG, NA], f32)
            nc.scalar.copy(out=w_sb[:], in_=w_ps[:])
            nc.sync.dma_start(out=w_d.rearrange("(i j) -> i j", i=NG), in_=w_sb[:])

            # ---- replicate w across all 128 partitions, beta too
            w_rep = cpool.tile([P, D], f32)
            nc.sync.dma_start(out=w_rep[:], in_=w_d[None, :].to_broadcast((P, D)))
            beta_sb = cpool.tile([P, 1], f32)
            nc.sync.dma_start(out=beta_sb[:], in_=beta_d[None, :].to_broadcast((P, 1)))

            y_sb = cpool.tile([P, N_TILES], f32)

            # row p*N_TILES + t  ->  partition p, column t
            x_v = x_d.rearrange("(p t) c -> p t c", p=P)
            y_v = y_d.rearrange("(p t) -> p t", p=P)

            for c in range(N_CHUNKS):
                xt = xpool.tile([P, T, D], f32)
                nc.sync.dma_start(out=xt[:], in_=x_v[:, c * T : (c + 1) * T, :])
                for k in range(T):
                    col = c * T + k
                    sc = scpool.tile([P, D], f32)
                    nc.vector.scalar_tensor_tensor(
                        out=sc[:],
                        in0=xt[:, k, :],
                        scalar=1.0,
                        in1=w_rep[:],
                        op0=mult,
                        op1=mult,
                        accum_out=y_sb[:, col : col + 1],
                    )

            nc.vector.tensor_scalar_add(out=y_sb[:], in0=y_sb[:], scalar1=beta_sb[:])
            nc.sync.dma_start(out=y_v, in_=y_sb[:])

    nc.compile()
    return nc


def kernel(x, beta_0, gamma, alpha):
    from concourse.bass_utils import run_bass_kernel_spmd

    if "nc" not in _CACHE:
        _CACHE["nc"] = _build()
    nc = _CACHE["nc"]

    x = np.ascontiguousarray(np.asarray(x, dtype=np.float32)).reshape(N_TOTAL, D)
    gamma_np = np.ascontiguousarray(np.asarray(gamma, dtype=np.float32))
    alpha_np = np.ascontiguousarray(np.asarray(alpha, dtype=np.float32))
    beta_np = np.asarray(beta_0, dtype=np.float32).reshape(1)

    in_maps = [
        {
            "x": x[i * N_PER_CORE : (i + 1) * N_PER_CORE],
            "gamma": gamma_np,
            "alpha": alpha_np,
            "beta": beta_np,
        }
        for i in range(N_CORES)
    ]

    res = run_bass_kernel_spmd(nc, in_maps, list(range(N_CORES)))
    y = np.concatenate([res.results[i]["y"] for i in range(N_CORES)])
    return y.astype(np.float32)


# revision 8
# speedup vs baseline: 1.5008x; 1.5008x over previous
"""CP/PARAFAC bilinear regression kernel for Trainium2 (8 NeuronCores).

Computes y[n] = beta_0 + sum_{i,j} x[n,i,j] * w[i,j],  w = gamma^T @ alpha.

Data-parallel over the batch axis: each of the 8 cores gets 16384 rows of x.

The reduction is HBM-bandwidth-bound, so the kernel is built around the
tensor engine (which has its own SBUF ports and leaves DMA + DVE free):

- Host side: x is cast to fp16 and laid out "planar transposed" per core:
  xt[c, k, j] = x_row[perm(j), c*128 + k], features padded 448->512 so each
  chunk c holds 128 features on 128 SBUF partitions. perm(j) (a 128x128
  transpose of the row index) is chosen so the OUTPUT lands contiguously.
- Device side: w = gamma^T @ alpha is computed in fp32 on the PE, split
  into an fp16 (hi, lo) pair for compensated fp16 matmuls. The main loop
  is pure DMA (16 x 1 MiB loads) + PE: per 128-row tile, 4 matmuls
  (lhsT = x^T chunk [128k x 128rows] fp16 stationary, rhs = w_hl[128k x 2])
  accumulate into one PSUM tile [128, 128, 2] (fp32). The epilogue folds
  hi+lo with one tensor_reduce, adds beta_0, and DMAs [128,128] out
  contiguously. The vector engine does no per-element work in the loop.

Accuracy: only the fp16 quantization of x contributes (~3e-4 scale-rel);
w is recovered to ~fp32 precision by the hi/lo split and PSUM accumulates
in fp32.
"""

import numpy as np

N_TOTAL = 131072
N_CORES = 8
N_PER_CORE = N_TOTAL // N_CORES  # 16384
NG = 7
NA = 64
RANK = 64
D = NG * NA  # 448
DP = 512  # padded feature dim
NC_CHUNK = DP // 128  # 4 feature chunks
P = 128
N_TILES = N_PER_CORE // P  # 128 row-tiles
GROUP_ROWS = 4096  # rows per DMA group (1 MiB per plane DMA)
N_GROUPS = N_PER_CORE // GROUP_ROWS  # 4
TILES_PER_GROUP = GROUP_ROWS // P  # 32

_CACHE = {}


def _build():
    from concourse import bacc, mybir, tile

    f32 = mybir.dt.float32
    f16 = mybir.dt.float16

    nc = bacc.Bacc("TRN2", target_bir_lowering=False, debug=False)

    xt_d = nc.dram_tensor(
        "xt", [NC_CHUNK, P, N_PER_CORE], f16, kind="ExternalInput"
    ).ap()
    gamma_d = nc.dram_tensor("gamma", [RANK, NG], f32, kind="ExternalInput").ap()
    alpha_d = nc.dram_tensor("alpha", [RANK, NA], f32, kind="ExternalInput").ap()
    beta_d = nc.dram_tensor("beta", [1], f32, kind="ExternalInput").ap()
    y_d = nc.dram_tensor("y", [N_PER_CORE], f32, kind="ExternalOutput").ap()
    w_d = nc.dram_tensor("w_scratch", [D], f32).ap()

    sub = mybir.AluOpType.subtract
    add = mybir.AluOpType.add

    with tile.TileContext(nc) as tc:
        with (
            tc.tile_pool(name="const", bufs=1) as cpool,
            tc.tile_pool(name="xp", bufs=3) as xpool,
            tc.tile_pool(name="ps", bufs=1, space="PSUM") as pspool,
            tc.tile_pool(name="pw", bufs=1, space="PSUM") as pwpool,
        ):
            # ---- w = gamma^T @ alpha on the PE (K = RANK = 64), fp32
            g_sb = cpool.tile([RANK, NG], f32)
            a_sb = cpool.tile([RANK, NA], f32)
            nc.sync.dma_start(out=g_sb[:], in_=gamma_d[:, :])
            nc.sync.dma_start(out=a_sb[:], in_=alpha_d[:, :])
            # stage through DVE so the PE matmul needs only ONE sem wait
            ga_sb = cpool.tile([RANK, NG + NA], f32)
            nc.vector.tensor_copy(out=ga_sb[:, :NG], in_=g_sb[:])
            nc.vector.tensor_copy(out=ga_sb[:, NG:], in_=a_sb[:])
            w_ps = pwpool.tile([NG, NA], f32)
            nc.tensor.matmul(
                w_ps[:], ga_sb[:, :NG], ga_sb[:, NG:], start=True, stop=True
            )
            w_sb = cpool.tile([NG, NA], f32)
            nc.scalar.copy(out=w_sb[:], in_=w_ps[:])
            nc.sync.dma_start(out=w_d.rearrange("(i j) -> i j", i=NG), in_=w_sb[:])

            # ---- bounce w back as [128 feats, 4 chunks] (flat f = c*128 + k)
            w32 = cpool.tile([P, NC_CHUNK], f32)
            nc.vector.memset(w32[:], 0.0)
            # chunks 0..2 are full 128-feature columns; chunk 3 has 64 real rows
            nc.sync.dma_start(
                out=w32[:, : NC_CHUNK - 1],
                in_=w_d[: 3 * P].rearrange("(c k) -> k c", c=NC_CHUNK - 1),
            )
            nc.sync.dma_start(
                out=w32[: D - 3 * P, NC_CHUNK - 1 :],
                in_=w_d[3 * P :][:, None],
            )

            # ---- split w into fp16 hi + lo (compensated precision)
            w_hi = cpool.tile([P, NC_CHUNK], f16)
            nc.vector.tensor_copy(out=w_hi[:], in_=w32[:])
            w_hi32 = cpool.tile([P, NC_CHUNK], f32)
            nc.vector.tensor_copy(out=w_hi32[:], in_=w_hi[:])
            w_lo32 = cpool.tile([P, NC_CHUNK], f32)
            nc.vector.tensor_tensor(
                out=w_lo32[:], in0=w32[:], in1=w_hi32[:], op=sub
            )
            w_hl = cpool.tile([P, NC_CHUNK, 2], f16)
            nc.vector.tensor_copy(out=w_hl[:, :, 0], in_=w_hi[:])
            nc.vector.tensor_copy(out=w_hl[:, :, 1], in_=w_lo32[:])

            beta_sb = cpool.tile([P, 1], f32)
            nc.sync.dma_start(out=beta_sb[:], in_=beta_d[None, :].to_broadcast((P, 1)))

            # ---- main loop: DMA planes + PE matmuls into one PSUM tile
            psum_y = pspool.tile([P, N_TILES, 2], f32)
            for g in range(N_GROUPS):
                xt = xpool.tile([P, NC_CHUNK, GROUP_ROWS], f16)
                for c in range(NC_CHUNK):
                    nc.sync.dma_start(
                        out=xt[:, c, :],
                        in_=xt_d[c, :, g * GROUP_ROWS : (g + 1) * GROUP_ROWS],
                    )
                for t in range(TILES_PER_GROUP):
                    tg = g * TILES_PER_GROUP + t
                    for c in range(NC_CHUNK):
                        nc.tensor.matmul(
                            psum_y[:, tg, :],
                            xt[:, c, t * P : (t + 1) * P],
                            w_hl[:, c, :],
                            start=(c == 0),
                            stop=(c == NC_CHUNK - 1),
                        )

            # ---- epilogue: fold hi+lo, add beta, store
            y_sb = cpool.tile([P, N_TILES], f32)
            nc.vector.tensor_reduce(
                out=y_sb[:], in_=psum_y[:], axis=mybir.AxisListType.X, op=add
            )
            nc.vector.tensor_scalar_add(out=y_sb[:], in0=y_sb[:], scalar1=beta_sb[:])
            nc.sync.dma_start(out=y_d.rearrange("(p t) -> p t", p=P), in_=y_sb[:])

    nc.compile()
    return nc


def _prep_x(x):
    """Full x [131072, 7, 64] f32 -> per-core planar fp16 [4, 128, 16384].

    Device column j of core i maps to global row i*16384 + perm(j) with
    perm(j) = (j % 128) * 128 + j // 128, so that PSUM partition m, tile T
    ends up holding y[m*128 + T] and the output DMA is contiguous.
    """
    xf = np.asarray(x, dtype=np.float32).reshape(N_TOTAL, D)
    out = []
    for i in range(N_CORES):
        a = xf[i * N_PER_CORE : (i + 1) * N_PER_CORE]
        # row permutation: j-th device column <- row (j%128)*128 + j//128
        a = a.reshape(P, N_TILES, D).swapaxes(0, 1).reshape(N_PER_CORE, D)
        ap = np.zeros((N_PER_CORE, DP), dtype=np.float16)
        ap[:, :D] = a
        out.append(np.ascontiguousarray(ap.T).reshape(NC_CHUNK, P, N_PER_CORE))
    return out


def _make_in_maps(x, beta_0, gamma, alpha):
    xt_shards = _prep_x(x)
    gamma_np = np.ascontiguousarray(np.asarray(gamma, dtype=np.float32))
    alpha_np = np.ascontiguousarray(np.asarray(alpha, dtype=np.float32))
    beta_np = np.asarray(beta_0, dtype=np.float32).reshape(1)
    return [
        {
            "xt": xt_shards[i],
            "gamma": gamma_np,
            "alpha": alpha_np,
            "beta": beta_np,
        }
        for i in range(N_CORES)
    ]


def kernel(x, beta_0, gamma, alpha):
    from concourse.bass_utils import run_bass_kernel_spmd

    if "nc" not in _CACHE:
        _CACHE["nc"] = _build()
    nc = _CACHE["nc"]

    in_maps = _make_in_maps(x, beta_0, gamma, alpha)
    res = run_bass_kernel_spmd(nc, in_maps, list(range(N_CORES)))
    y = np.concatenate([res.results[i]["y"] for i in range(N_CORES)])
    return y.astype(np.float32)
